# revision 6
# baseline (speedup 1.0000x reference)
"""Trainium2 Bass kernel for additive attention (nn_AdditiveAttention).

Reference computation (per batch b):
    q_proj = query @ W1_w.T + W1_b                      # [D]
    v_proj = values @ W2_w.T + W2_b                     # [T, D]
    scores = tanh(q_proj + v_proj) @ v                  # [T]
    weights = softmax(scores)                           # [T]
    out    = weights @ values                           # [E]

Sharding: data-parallel over batch B=32 across 8 NeuronCores (4 batches/core).

V3 design (vs the PE-transpose baseline): `values` is streamed from HBM
TWICE as bf16 per group of 2048 timesteps -- once in natural [t, e] layout
(numerator operand) and once through the DMA XBAR transpose into [e, t]
layout (v_proj moving operand). This removes all 1024 PE identity-transpose
matmuls and their PSUM->SBUF copies; the PE only runs v_proj, scores,
4 tiny score-row transposes and the numerator. Engine budget per core:
PE ~200us, DMA 2x32MB ~178us, ACT (tanh) ~100us.

Per (batch, group of 2048 t):
  - DMA natural  vg [128t, 16, 512e] bf16   (one 2MB DMA)
  - DMA transposed vt [128e, 4, 2048t] bf16 (one 2MB XBAR-transpose DMA)
  - per super s (512 t): 2 psum tiles [128d, 512t] = W2ed-chunk.T @ vt-chunk
    (4 accumulating MMs each); ACT tanh w/ per-partition bias -> th bf16;
    2 score MMs (lhsT=v) -> row s of a [4, 512] psum tile
  - softmax: one DVE copy [4,512] -> SBUF; 4 PE mini-transposes [4,128] ->
    [128,4] score columns; DVE free-max + GPSIMD partition all-reduce ->
    group max M; ACT exp(s-M) -> weight cols [128,16]; DVE row-sum +
    GPSIMD -> den
  - numerator: psum [1, 512e] += wg-col.T @ vg-tile (16 MMs), deferred into
    the next group's v_proj phase so the PE always has queued work
  - per (batch, group) output row: [num(512), den, M] -> host combine

All matmul operands are bf16 (~4e-3 rel err; harness gate is 2e-2).
"""

import os
import sys
import time

import numpy as np

for _p in ("/opt/trn_rl_repo",):
    if _p not in sys.path and os.path.isdir(_p):
        sys.path.insert(0, _p)

# Problem shapes (hardcoded per contract)
B, T, E, D = 32, 8192, 512, 256
N_CORES = 8
B_LOC = B // N_CORES          # 4 batches per core
P = 128
TSUP = 512                    # timesteps per super tile
JSUB = TSUP // P              # 4 basic 128-t subtiles per super
SUP_PER_GROUP = 4             # supers per softmax group
T_GROUP = TSUP * SUP_PER_GROUP  # 2048
EC = E // P                   # 4 e-chunks
DC = D // P                   # 2 d-chunks
NB = SUP_PER_GROUP * JSUB     # 16 basic tiles (numerator cols) per group
OUTW = E + 2                  # num[512], den, M

LAST_RESULT = None            # BassKernelResults of the most recent run


def build_bass(t_loc=T, b_loc=B_LOC, vpool_bufs=3, repeat=1, loop_n=1):
    """Build the Bass module (same SPMD program for every core)."""
    import concourse.bacc as bacc
    import concourse.tile as tile
    from concourse import mybir

    f32 = mybir.dt.float32
    dtm = mybir.dt.bfloat16

    n_groups = t_loc // T_GROUP
    assert t_loc % T_GROUP == 0

    nc = bacc.Bacc("TRN2", target_bir_lowering=False, debug=False,
                   num_devices=N_CORES)
    vals = nc.dram_tensor("values", [b_loc, t_loc, E], dtm,
                          kind="ExternalInput").ap()
    w2ed_d = nc.dram_tensor("w2ed", [E, D], dtm, kind="ExternalInput").ap()
    cb_d = nc.dram_tensor("cb", [D, b_loc], f32, kind="ExternalInput").ap()
    v_d = nc.dram_tensor("vcol", [D, 1], dtm, kind="ExternalInput").ap()
    id32_d = nc.dram_tensor("ident32", [P, P], mybir.dt.float32r,
                            kind="ExternalInput").ap()
    outp = nc.dram_tensor("out_parts", [b_loc, n_groups, OUTW], f32,
                          kind="ExternalOutput").ap()

    with tile.TileContext(nc) as tc:
        _emit(tc, vals, w2ed_d, cb_d, v_d, id32_d, outp, b_loc,
              n_groups, vpool_bufs, repeat, loop_n, dtm)
    nc.compile()
    return nc


def _emit(tc, vals, w2ed_d, cb_d, v_d, id32_d, outp, b_loc,
          n_groups, vpool_bufs, repeat, loop_n, dtm):
    from contextlib import ExitStack

    from concourse import bass_isa, mybir

    f32 = mybir.dt.float32
    f32r = mybir.dt.float32r
    Tanh = mybir.ActivationFunctionType.Tanh
    Exp = mybir.ActivationFunctionType.Exp
    X = mybir.AxisListType.X

    nc = tc.nc

    with ExitStack() as ctx:
        consts = ctx.enter_context(tc.tile_pool(name="consts", bufs=1))
        vpool = ctx.enter_context(tc.tile_pool(name="vpool", bufs=vpool_bufs))
        vtpool = ctx.enter_context(
            tc.tile_pool(name="vtpool", bufs=vpool_bufs))
        thpool = ctx.enter_context(tc.tile_pool(name="thpool", bufs=4))
        rowpool = ctx.enter_context(tc.tile_pool(name="rowpool", bufs=2))
        spool = ctx.enter_context(tc.tile_pool(name="spool", bufs=3))
        redpool = ctx.enter_context(tc.tile_pool(name="redpool", bufs=4))
        opool = ctx.enter_context(tc.tile_pool(name="opool", bufs=4))
        ps_vp = ctx.enter_context(
            tc.tile_pool(name="ps_vp", bufs=2, space="PSUM"))
        ps_sm = ctx.enter_context(
            tc.tile_pool(name="ps_sm", bufs=2, space="PSUM"))
        ps_s4 = ctx.enter_context(
            tc.tile_pool(name="ps_s4", bufs=2, space="PSUM"))
        ps_nm = ctx.enter_context(
            tc.tile_pool(name="ps_nm", bufs=2, space="PSUM"))

        # GPSIMD ucode library providing partition_all_reduce
        from concourse import library_config
        nc.gpsimd.load_library(library_config.mlp)

        # --- constants ---
        w2_sb = consts.tile([P, EC, D], dtm)
        nc.sync.dma_start(w2_sb, w2ed_d.rearrange("(c p) d -> p c d", p=P))
        cb_sb = consts.tile([P, DC, b_loc], f32)
        nc.sync.dma_start(cb_sb, cb_d.rearrange("(c p) b -> p c b", p=P))
        v_sb = consts.tile([P, DC, 1], dtm)
        nc.sync.dma_start(v_sb, v_d.rearrange("(c p) x -> p c x", p=P))
        id32_sb = consts.tile([P, P], f32r)
        nc.sync.dma_start(id32_sb, id32_d)

        num_state = {}

        def emit_numerator_half(p, k0, count):
            vg, wg, b, g, m_all, den_all, rep = p
            key = (rep, b, g)
            if key not in num_state:
                num_state[key] = ps_nm.tile([1, E], f32, tag="num",
                                            name=f"psn_{rep}_{b}_{g}")
            psn = num_state[key]
            for k in range(k0, k0 + count):
                tcn, s = divmod(k, SUP_PER_GROUP)
                col = tcn * SUP_PER_GROUP + s
                nc.tensor.matmul(
                    psn,
                    lhsT=wg[:, col:col + 1],
                    rhs=vg[:, s * JSUB + tcn, :],
                    start=(k == 0), stop=(k == NB - 1))

        def emit_numerator_tail(p):
            vg, wg, b, g, m_all, den_all, rep = p
            psn = num_state.pop((rep, b, g))
            osb = opool.tile([1, OUTW], f32, tag="osb",
                             name=f"osb_{rep}_{b}_{g}")
            nc.scalar.copy(osb[:, 0:E], psn)
            nc.vector.tensor_copy(osb[:, E:E + 1], den_all[0:1, :])
            nc.vector.tensor_copy(osb[:, E + 1:E + 2], m_all[0:1, :])
            nc.sync.dma_start(outp[b, g:g + 1, :], osb)

        def emit_numerator(p):
            emit_numerator_half(p, 0, NB)
            emit_numerator_tail(p)

        def body(rep):
          pending = None
          for b in range(b_loc):
            for g in range(n_groups):
                # ---------------- phase A: scores for this group ----------
                t0g = g * T_GROUP
                vg = vpool.tile([P, NB, TSUP], dtm, tag="vg",
                                name=f"vg_{rep}_{b}_{g}")
                nc.sync.dma_start(
                    vg,
                    vals[b, t0g:t0g + T_GROUP, :].rearrange(
                        "(sj p) e -> p sj e", p=P))
                # Transposed copy via the DMA XBAR: vt[p, c, t] =
                # values[t0g + t, c*128 + p]  (verified mapping).
                vt = vtpool.tile([P, EC, T_GROUP], dtm, tag="vt",
                                 name=f"vt_{rep}_{b}_{g}")
                nc.sync.dma_start(vt, vals[b, t0g:t0g + T_GROUP, :],
                                  transpose=True)

                # scores psum: two banks of 2 supers each, rows {0,32}
                # (PSUM matmul col-groups allow only {0,32,64}).
                pss_a = ps_sm.tile([64, TSUP], f32, tag="scrow",
                                   name=f"pssa_{rep}_{b}_{g}")
                pss_b = ps_sm.tile([64, TSUP], f32, tag="scrow",
                                   name=f"pssb_{rep}_{b}_{g}")

                def score_row(sp):
                    t = pss_a if sp < 2 else pss_b
                    r = 32 * (sp % 2)
                    return t[r:r + 1, :]

                ths_q = []
                for s in range(SUP_PER_GROUP):
                    ts0 = s * TSUP
                    ths = []
                    for dc in range(DC):
                        psv = ps_vp.tile([P, TSUP], f32, tag="psv",
                                         name=f"psv_{rep}_{b}_{g}_{s}_{dc}")
                        for c in range(EC):
                            nc.tensor.matmul(
                                psv,
                                lhsT=w2_sb[:, c, dc * P:(dc + 1) * P],
                                rhs=vt[:, c, ts0:ts0 + TSUP],
                                start=(c == 0), stop=(c == EC - 1))
                        th = thpool.tile([P, TSUP], dtm, tag="th",
                                         name=f"th_{rep}_{b}_{g}_{s}_{dc}")
                        nc.scalar.activation(th, psv, Tanh,
                                             bias=cb_sb[:, dc, b:b + 1])
                        ths.append(th)
                    ths_q.append(ths)
                    # Emit score MMs one super behind the v_proj MMs so the
                    # PE has independent queued work while tanh drains.
                    if s >= 1:
                        sp = s - 1
                        nc.tensor.matmul(score_row(sp),
                                         lhsT=v_sb[:, 0, :], rhs=ths_q[sp][0],
                                         start=True, stop=False)
                        nc.tensor.matmul(score_row(sp),
                                         lhsT=v_sb[:, 1, :], rhs=ths_q[sp][1],
                                         start=False, stop=True)
                # previous group's numerator fills the PE while the last
                # tanh drains
                if pending is not None:
                    emit_numerator_half(pending, 0, NB // 2)
                sp = SUP_PER_GROUP - 1
                nc.tensor.matmul(score_row(sp),
                                 lhsT=v_sb[:, 0, :], rhs=ths_q[sp][0],
                                 start=True, stop=False)
                if pending is not None:
                    emit_numerator_half(pending, NB // 2, NB // 2)
                    emit_numerator_tail(pending)
                    pending = None
                nc.tensor.matmul(score_row(sp),
                                 lhsT=v_sb[:, 1, :], rhs=ths_q[sp][1],
                                 start=False, stop=True)

                # ------------- scores rows -> columns ---------------------
                # Full 128x128 transpose; meaningful score rows sit at
                # partitions {0,32,64,96}, so cols {0,32,64,96} of the
                # transposed tile hold the per-basic score columns.
                srow = rowpool.tile([P, TSUP], f32r, tag="srow",
                                    name=f"srow_{rep}_{b}_{g}")
                nc.vector.tensor_copy(srow[0:64, :], pss_a)
                nc.vector.tensor_copy(srow[64:128, :], pss_b)
                sg = spool.tile([P, NB], f32, tag="sg",
                                name=f"sg_{rep}_{b}_{g}")
                for tcn in range(JSUB):
                    ps4 = ps_s4.tile([P, P], f32r, tag="s4",
                                     name=f"ps4_{rep}_{b}_{g}_{tcn}")
                    nc.tensor.transpose(
                        ps4,
                        srow[:, tcn * P:(tcn + 1) * P],
                        id32_sb)
                    nc.vector.tensor_copy(
                        sg[:, tcn * SUP_PER_GROUP:(tcn + 1) * SUP_PER_GROUP],
                        ps4.rearrange("p (s x) -> p s x", s=SUP_PER_GROUP)
                        [:, :, 0:1])

                # ------------- softmax pieces (max, exp, den) -------------
                m_part = redpool.tile([P, 1], f32, tag="mp",
                                      name=f"mp_{rep}_{b}_{g}")
                nc.vector.reduce_max(m_part, sg, axis=X)
                m_all = redpool.tile([P, 1], f32, tag="ma",
                                     name=f"ma_{rep}_{b}_{g}")
                nc.gpsimd.partition_all_reduce(
                    m_all, m_part, channels=P,
                    reduce_op=bass_isa.ReduceOp.max)
                negm = redpool.tile([P, 1], f32, tag="nm",
                                    name=f"nm_{rep}_{b}_{g}")
                nc.vector.tensor_scalar_mul(negm, m_all, -1.0)
                wg = spool.tile([P, NB], dtm, tag="wg",
                                name=f"wg_{rep}_{b}_{g}")
                nc.scalar.activation(wg, sg, Exp, bias=negm)
                wsum = redpool.tile([P, 1], f32, tag="ws",
                                    name=f"ws_{rep}_{b}_{g}")
                nc.vector.reduce_sum(wsum, wg, axis=X)
                den_all = redpool.tile([P, 1], f32, tag="da",
                                       name=f"da_{rep}_{b}_{g}")
                nc.gpsimd.partition_all_reduce(
                    den_all, wsum, channels=P,
                    reduce_op=bass_isa.ReduceOp.add)

                pending = (vg, wg, b, g, m_all, den_all, rep)
          emit_numerator(pending)

        if loop_n > 1:
            with tc.For_i(0, loop_n, 1):
                body(0)
        else:
            for rep in range(repeat):
                body(rep)


def host_prepare(values, query, v, W1_w, W1_b, W2_w, W2_b, b_loc=B_LOC,
                 n_cores=N_CORES):
    """Precompute tiny host-side tensors and build per-core input maps."""
    import ml_dtypes

    npm = ml_dtypes.bfloat16

    c = (query.astype(np.float32) @ W1_w.T.astype(np.float32)
         + W1_b + W2_b).astype(np.float32)          # [B, D]
    values_m = np.ascontiguousarray(np.asarray(values).astype(npm))
    w2ed = np.ascontiguousarray(np.asarray(W2_w).T.astype(npm))  # [E, D]
    vcol = np.ascontiguousarray(np.asarray(v).reshape(D, 1).astype(npm))
    ident32 = np.eye(P, dtype=np.float32)
    in_maps = []
    for k in range(n_cores):
        bsl = slice(k * b_loc, (k + 1) * b_loc)
        in_maps.append({
            "values": np.ascontiguousarray(values_m[bsl]),
            "w2ed": w2ed,
            "cb": np.ascontiguousarray(c[bsl].T),    # [D, b_loc]
            "vcol": vcol,
            "ident32": ident32,
        })
    return in_maps


def host_combine(results, b_loc=B_LOC, n_cores=N_CORES):
    """Combine per-(batch, group) partial softmax numerators/denominators."""
    out = np.zeros((n_cores * b_loc, E), np.float32)
    for k in range(n_cores):
        parts = np.asarray(results[k]["out_parts"])  # [b_loc, n_groups, 514]
        num = parts[..., :E].astype(np.float64)
        den = parts[..., E].astype(np.float64)
        M = parts[..., E + 1].astype(np.float64)
        Mb = M.max(axis=1, keepdims=True)
        sc = np.exp(M - Mb)                          # [b_loc, n_groups]
        o = (num * sc[..., None]).sum(1) / (den * sc).sum(1)[:, None]
        out[k * b_loc:(k + 1) * b_loc] = o.astype(np.float32)
    return out


_NC_CACHE = None


def kernel(values, query, v, W1_w, W1_b, W2_w, W2_b):
    global _NC_CACHE, LAST_RESULT
    from concourse.bass_utils import run_bass_kernel_spmd

    in_maps = host_prepare(values, query, v, W1_w, W1_b, W2_w, W2_b)
    if _NC_CACHE is None:
        _NC_CACHE = build_bass()
    trace = bool(int(os.environ.get("KERNEL_TRACE", "0")))
    LAST_RESULT = run_bass_kernel_spmd(
        _NC_CACHE, in_maps, list(range(N_CORES)), trace=trace)
    return host_combine(LAST_RESULT.results)


if __name__ == "__main__":
    rng = np.random.default_rng(0)
    inputs = {
        "values": rng.standard_normal((B, T, E), dtype=np.float32),
        "query": rng.standard_normal((B, D), dtype=np.float32),
        "v": rng.random(D, dtype=np.float32),
        "W1_w": rng.standard_normal((D, D), dtype=np.float32) * 0.06,
        "W1_b": rng.standard_normal(D, dtype=np.float32) * 0.06,
        "W2_w": rng.standard_normal((D, E), dtype=np.float32) * 0.04,
        "W2_b": rng.standard_normal(D, dtype=np.float32) * 0.04,
    }
    t0 = time.time()
    out = kernel(**inputs)
    print("kernel done in", time.time() - t0, "s", out.shape, out.dtype)


# revision 7
# speedup vs baseline: 1.3612x; 1.3612x over previous
"""Trainium2 Bass kernel for additive attention (nn_AdditiveAttention).

Reference computation (per batch b):
    q_proj = query @ W1_w.T + W1_b                      # [D]
    v_proj = values @ W2_w.T + W2_b                     # [T, D]
    scores = tanh(q_proj + v_proj) @ v                  # [T]
    weights = softmax(scores)                           # [T]
    out    = weights @ values                           # [E]

Sharding: data-parallel over batch B=32 across 8 NeuronCores (4 batches/core).

V3 design (vs the PE-transpose baseline): `values` is streamed from HBM
TWICE as bf16 per group of 2048 timesteps -- once in natural [t, e] layout
(numerator operand) and once through the DMA XBAR transpose into [e, t]
layout (v_proj moving operand). This removes all 1024 PE identity-transpose
matmuls and their PSUM->SBUF copies; the PE only runs v_proj, scores,
4 tiny score-row transposes and the numerator. Engine budget per core:
PE ~200us, DMA 2x32MB ~178us, ACT (tanh) ~100us.

Per (batch, group of 2048 t):
  - DMA natural  vg [128t, 16, 512e] bf16   (one 2MB DMA)
  - DMA transposed vt [128e, 4, 2048t] bf16 (one 2MB XBAR-transpose DMA)
  - per super s (512 t): 2 psum tiles [128d, 512t] = W2ed-chunk.T @ vt-chunk
    (4 accumulating MMs each); ACT tanh w/ per-partition bias -> th bf16;
    2 score MMs (lhsT=v) -> row s of a [4, 512] psum tile
  - softmax: one DVE copy [4,512] -> SBUF; 4 PE mini-transposes [4,128] ->
    [128,4] score columns; DVE free-max + GPSIMD partition all-reduce ->
    group max M; ACT exp(s-M) -> weight cols [128,16]; DVE row-sum +
    GPSIMD -> den
  - numerator: psum [1, 512e] += wg-col.T @ vg-tile (16 MMs), deferred into
    the next group's v_proj phase so the PE always has queued work
  - per (batch, group) output row: [num(512), den, M] -> host combine

All matmul operands are bf16 (~4e-3 rel err; harness gate is 2e-2).
"""

import os
import sys
import time

import numpy as np

for _p in ("/opt/trn_rl_repo",):
    if _p not in sys.path and os.path.isdir(_p):
        sys.path.insert(0, _p)

# Problem shapes (hardcoded per contract)
B, T, E, D = 32, 8192, 512, 256
N_CORES = 8
B_LOC = B // N_CORES          # 4 batches per core
P = 128
TSUP = 512                    # timesteps per super tile
JSUB = TSUP // P              # 4 basic 128-t subtiles per super
SUP_PER_GROUP = 4             # supers per softmax group
T_GROUP = TSUP * SUP_PER_GROUP  # 2048
EC = E // P                   # 4 e-chunks
DC = D // P                   # 2 d-chunks
NB = SUP_PER_GROUP * JSUB     # 16 basic tiles (numerator cols) per group
OUTW = E + 2                  # num[512], den, M

LAST_RESULT = None            # BassKernelResults of the most recent run


def build_bass(t_loc=T, b_loc=B_LOC, vpool_bufs=3, repeat=1, loop_n=1):
    """Build the Bass module (same SPMD program for every core)."""
    import concourse.bacc as bacc
    import concourse.tile as tile
    from concourse import mybir

    f32 = mybir.dt.float32
    dtm = mybir.dt.bfloat16

    n_groups = t_loc // T_GROUP
    assert t_loc % T_GROUP == 0

    nc = bacc.Bacc("TRN2", target_bir_lowering=False, debug=False,
                   num_devices=N_CORES)
    vals = nc.dram_tensor("values", [b_loc, t_loc, E], dtm,
                          kind="ExternalInput").ap()
    valsT = nc.dram_tensor("valuesT", [b_loc, E, t_loc], dtm,
                           kind="ExternalInput").ap()
    w2ed_d = nc.dram_tensor("w2ed", [E, D], dtm, kind="ExternalInput").ap()
    cb_d = nc.dram_tensor("cb", [D, b_loc], f32, kind="ExternalInput").ap()
    v_d = nc.dram_tensor("vcol", [D, 1], dtm, kind="ExternalInput").ap()
    id32_d = nc.dram_tensor("ident32", [P, P], mybir.dt.float32r,
                            kind="ExternalInput").ap()
    outp = nc.dram_tensor("out_parts", [b_loc, n_groups, OUTW], f32,
                          kind="ExternalOutput").ap()

    with tile.TileContext(nc) as tc:
        _emit(tc, vals, valsT, w2ed_d, cb_d, v_d, id32_d, outp, b_loc,
              n_groups, vpool_bufs, repeat, loop_n, dtm)
    nc.compile()
    return nc


def _emit(tc, vals, valsT, w2ed_d, cb_d, v_d, id32_d, outp, b_loc,
          n_groups, vpool_bufs, repeat, loop_n, dtm):
    from contextlib import ExitStack

    from concourse import bass_isa, mybir

    f32 = mybir.dt.float32
    f32r = mybir.dt.float32r
    Tanh = mybir.ActivationFunctionType.Tanh
    Exp = mybir.ActivationFunctionType.Exp
    X = mybir.AxisListType.X

    nc = tc.nc

    with ExitStack() as ctx:
        consts = ctx.enter_context(tc.tile_pool(name="consts", bufs=1))
        vpool = ctx.enter_context(tc.tile_pool(name="vpool", bufs=vpool_bufs))
        vtpool = ctx.enter_context(
            tc.tile_pool(name="vtpool", bufs=vpool_bufs))
        thpool = ctx.enter_context(tc.tile_pool(name="thpool", bufs=4))
        rowpool = ctx.enter_context(tc.tile_pool(name="rowpool", bufs=2))
        spool = ctx.enter_context(tc.tile_pool(name="spool", bufs=3))
        redpool = ctx.enter_context(tc.tile_pool(name="redpool", bufs=4))
        opool = ctx.enter_context(tc.tile_pool(name="opool", bufs=4))
        ps_vp = ctx.enter_context(
            tc.tile_pool(name="ps_vp", bufs=2, space="PSUM"))
        ps_sm = ctx.enter_context(
            tc.tile_pool(name="ps_sm", bufs=2, space="PSUM"))
        ps_s4 = ctx.enter_context(
            tc.tile_pool(name="ps_s4", bufs=2, space="PSUM"))
        ps_nm = ctx.enter_context(
            tc.tile_pool(name="ps_nm", bufs=2, space="PSUM"))

        # GPSIMD ucode library providing partition_all_reduce
        from concourse import library_config
        nc.gpsimd.load_library(library_config.mlp)

        # --- constants ---
        w2_sb = consts.tile([P, EC, D], dtm)
        nc.sync.dma_start(w2_sb, w2ed_d.rearrange("(c p) d -> p c d", p=P))
        cb_sb = consts.tile([P, DC, b_loc], f32)
        nc.sync.dma_start(cb_sb, cb_d.rearrange("(c p) b -> p c b", p=P))
        v_sb = consts.tile([P, DC, 1], dtm)
        nc.sync.dma_start(v_sb, v_d.rearrange("(c p) x -> p c x", p=P))
        id32_sb = consts.tile([P, P], f32r)
        nc.sync.dma_start(id32_sb, id32_d)

        num_state = {}

        def emit_numerator_half(p, k0, count):
            vg, wg, b, g, m_all, den_all, rep = p
            key = (rep, b, g)
            if key not in num_state:
                num_state[key] = ps_nm.tile([1, E], f32, tag="num",
                                            name=f"psn_{rep}_{b}_{g}")
            psn = num_state[key]
            for k in range(k0, k0 + count):
                tcn, s = divmod(k, SUP_PER_GROUP)
                col = tcn * SUP_PER_GROUP + s
                nc.tensor.matmul(
                    psn,
                    lhsT=wg[:, col:col + 1],
                    rhs=vg[:, s * JSUB + tcn, :],
                    start=(k == 0), stop=(k == NB - 1))

        def emit_numerator_tail(p):
            vg, wg, b, g, m_all, den_all, rep = p
            psn = num_state.pop((rep, b, g))
            osb = opool.tile([1, OUTW], f32, tag="osb",
                             name=f"osb_{rep}_{b}_{g}")
            nc.scalar.copy(osb[:, 0:E], psn)
            nc.vector.tensor_copy(osb[:, E:E + 1], den_all[0:1, :])
            nc.vector.tensor_copy(osb[:, E + 1:E + 2], m_all[0:1, :])
            nc.sync.dma_start(outp[b, g:g + 1, :], osb)

        def emit_numerator(p):
            emit_numerator_half(p, 0, NB)
            emit_numerator_tail(p)

        def body(rep):
          pending = None
          for b in range(b_loc):
            for g in range(n_groups):
                # ---------------- phase A: scores for this group ----------
                t0g = g * T_GROUP
                vg = vpool.tile([P, NB, TSUP], dtm, tag="vg",
                                name=f"vg_{rep}_{b}_{g}")
                nc.sync.dma_start(
                    vg,
                    vals[b, t0g:t0g + T_GROUP, :].rearrange(
                        "(sj p) e -> p sj e", p=P))
                # Transposed copy, host-pretransposed: plain contiguous
                # load (the DMA XBAR transpose path measures only ~166 GB/s
                # vs ~392 GB/s for straight loads). vt[p, c, t] =
                # valuesT[b, c*128 + p, t0g + t].
                vt = vtpool.tile([P, EC, T_GROUP], dtm, tag="vt",
                                 name=f"vt_{rep}_{b}_{g}")
                nc.sync.dma_start(
                    vt,
                    valsT[b, :, t0g:t0g + T_GROUP].rearrange(
                        "(c p) t -> p c t", p=P))

                # scores psum: two banks of 2 supers each, rows {0,32}
                # (PSUM matmul col-groups allow only {0,32,64}).
                pss_a = ps_sm.tile([64, TSUP], f32, tag="scrow",
                                   name=f"pssa_{rep}_{b}_{g}")
                pss_b = ps_sm.tile([64, TSUP], f32, tag="scrow",
                                   name=f"pssb_{rep}_{b}_{g}")

                def score_row(sp):
                    t = pss_a if sp < 2 else pss_b
                    r = 32 * (sp % 2)
                    return t[r:r + 1, :]

                ths_q = []
                for s in range(SUP_PER_GROUP):
                    ts0 = s * TSUP
                    ths = []
                    for dc in range(DC):
                        psv = ps_vp.tile([P, TSUP], f32, tag="psv",
                                         name=f"psv_{rep}_{b}_{g}_{s}_{dc}")
                        for c in range(EC):
                            nc.tensor.matmul(
                                psv,
                                lhsT=w2_sb[:, c, dc * P:(dc + 1) * P],
                                rhs=vt[:, c, ts0:ts0 + TSUP],
                                start=(c == 0), stop=(c == EC - 1))
                        th = thpool.tile([P, TSUP], dtm, tag="th",
                                         name=f"th_{rep}_{b}_{g}_{s}_{dc}")
                        nc.scalar.activation(th, psv, Tanh,
                                             bias=cb_sb[:, dc, b:b + 1])
                        ths.append(th)
                    ths_q.append(ths)
                    # Emit score MMs one super behind the v_proj MMs so the
                    # PE has independent queued work while tanh drains.
                    if s >= 1:
                        sp = s - 1
                        nc.tensor.matmul(score_row(sp),
                                         lhsT=v_sb[:, 0, :], rhs=ths_q[sp][0],
                                         start=True, stop=False)
                        nc.tensor.matmul(score_row(sp),
                                         lhsT=v_sb[:, 1, :], rhs=ths_q[sp][1],
                                         start=False, stop=True)
                # previous group's numerator fills the PE while the last
                # tanh drains
                if pending is not None:
                    emit_numerator_half(pending, 0, NB // 2)
                sp = SUP_PER_GROUP - 1
                nc.tensor.matmul(score_row(sp),
                                 lhsT=v_sb[:, 0, :], rhs=ths_q[sp][0],
                                 start=True, stop=False)
                if pending is not None:
                    emit_numerator_half(pending, NB // 2, NB // 2)
                    emit_numerator_tail(pending)
                    pending = None
                nc.tensor.matmul(score_row(sp),
                                 lhsT=v_sb[:, 1, :], rhs=ths_q[sp][1],
                                 start=False, stop=True)

                # ------------- scores rows -> columns ---------------------
                # Full 128x128 transpose; meaningful score rows sit at
                # partitions {0,32,64,96}, so cols {0,32,64,96} of the
                # transposed tile hold the per-basic score columns.
                srow = rowpool.tile([P, TSUP], f32r, tag="srow",
                                    name=f"srow_{rep}_{b}_{g}")
                nc.vector.tensor_copy(srow[0:64, :], pss_a)
                nc.vector.tensor_copy(srow[64:128, :], pss_b)
                sg = spool.tile([P, NB], f32, tag="sg",
                                name=f"sg_{rep}_{b}_{g}")
                for tcn in range(JSUB):
                    ps4 = ps_s4.tile([P, P], f32r, tag="s4",
                                     name=f"ps4_{rep}_{b}_{g}_{tcn}")
                    nc.tensor.transpose(
                        ps4,
                        srow[:, tcn * P:(tcn + 1) * P],
                        id32_sb)
                    nc.vector.tensor_copy(
                        sg[:, tcn * SUP_PER_GROUP:(tcn + 1) * SUP_PER_GROUP],
                        ps4.rearrange("p (s x) -> p s x", s=SUP_PER_GROUP)
                        [:, :, 0:1])

                # ------------- softmax pieces (max, exp, den) -------------
                m_part = redpool.tile([P, 1], f32, tag="mp",
                                      name=f"mp_{rep}_{b}_{g}")
                nc.vector.reduce_max(m_part, sg, axis=X)
                m_all = redpool.tile([P, 1], f32, tag="ma",
                                     name=f"ma_{rep}_{b}_{g}")
                nc.gpsimd.partition_all_reduce(
                    m_all, m_part, channels=P,
                    reduce_op=bass_isa.ReduceOp.max)
                negm = redpool.tile([P, 1], f32, tag="nm",
                                    name=f"nm_{rep}_{b}_{g}")
                nc.vector.tensor_scalar_mul(negm, m_all, -1.0)
                wg = spool.tile([P, NB], dtm, tag="wg",
                                name=f"wg_{rep}_{b}_{g}")
                nc.scalar.activation(wg, sg, Exp, bias=negm)
                wsum = redpool.tile([P, 1], f32, tag="ws",
                                    name=f"ws_{rep}_{b}_{g}")
                nc.vector.reduce_sum(wsum, wg, axis=X)
                den_all = redpool.tile([P, 1], f32, tag="da",
                                       name=f"da_{rep}_{b}_{g}")
                nc.gpsimd.partition_all_reduce(
                    den_all, wsum, channels=P,
                    reduce_op=bass_isa.ReduceOp.add)

                pending = (vg, wg, b, g, m_all, den_all, rep)
          emit_numerator(pending)

        if loop_n > 1:
            with tc.For_i(0, loop_n, 1):
                body(0)
        else:
            for rep in range(repeat):
                body(rep)


def host_prepare(values, query, v, W1_w, W1_b, W2_w, W2_b, b_loc=B_LOC,
                 n_cores=N_CORES):
    """Precompute tiny host-side tensors and build per-core input maps."""
    import ml_dtypes

    npm = ml_dtypes.bfloat16

    c = (query.astype(np.float32) @ W1_w.T.astype(np.float32)
         + W1_b + W2_b).astype(np.float32)          # [B, D]
    values_m = np.ascontiguousarray(np.asarray(values).astype(npm))
    values_t = np.ascontiguousarray(values_m.transpose(0, 2, 1))
    w2ed = np.ascontiguousarray(np.asarray(W2_w).T.astype(npm))  # [E, D]
    vcol = np.ascontiguousarray(np.asarray(v).reshape(D, 1).astype(npm))
    ident32 = np.eye(P, dtype=np.float32)
    in_maps = []
    for k in range(n_cores):
        bsl = slice(k * b_loc, (k + 1) * b_loc)
        in_maps.append({
            "values": np.ascontiguousarray(values_m[bsl]),
            "valuesT": np.ascontiguousarray(values_t[bsl]),
            "w2ed": w2ed,
            "cb": np.ascontiguousarray(c[bsl].T),    # [D, b_loc]
            "vcol": vcol,
            "ident32": ident32,
        })
    return in_maps


def host_combine(results, b_loc=B_LOC, n_cores=N_CORES):
    """Combine per-(batch, group) partial softmax numerators/denominators."""
    out = np.zeros((n_cores * b_loc, E), np.float32)
    for k in range(n_cores):
        parts = np.asarray(results[k]["out_parts"])  # [b_loc, n_groups, 514]
        num = parts[..., :E].astype(np.float64)
        den = parts[..., E].astype(np.float64)
        M = parts[..., E + 1].astype(np.float64)
        Mb = M.max(axis=1, keepdims=True)
        sc = np.exp(M - Mb)                          # [b_loc, n_groups]
        o = (num * sc[..., None]).sum(1) / (den * sc).sum(1)[:, None]
        out[k * b_loc:(k + 1) * b_loc] = o.astype(np.float32)
    return out


_NC_CACHE = None


def kernel(values, query, v, W1_w, W1_b, W2_w, W2_b):
    global _NC_CACHE, LAST_RESULT
    from concourse.bass_utils import run_bass_kernel_spmd

    in_maps = host_prepare(values, query, v, W1_w, W1_b, W2_w, W2_b)
    if _NC_CACHE is None:
        _NC_CACHE = build_bass()
    trace = bool(int(os.environ.get("KERNEL_TRACE", "0")))
    LAST_RESULT = run_bass_kernel_spmd(
        _NC_CACHE, in_maps, list(range(N_CORES)), trace=trace)
    return host_combine(LAST_RESULT.results)


if __name__ == "__main__":
    rng = np.random.default_rng(0)
    inputs = {
        "values": rng.standard_normal((B, T, E), dtype=np.float32),
        "query": rng.standard_normal((B, D), dtype=np.float32),
        "v": rng.random(D, dtype=np.float32),
        "W1_w": rng.standard_normal((D, D), dtype=np.float32) * 0.06,
        "W1_b": rng.standard_normal(D, dtype=np.float32) * 0.06,
        "W2_w": rng.standard_normal((D, E), dtype=np.float32) * 0.04,
        "W2_b": rng.standard_normal(D, dtype=np.float32) * 0.04,
    }
    t0 = time.time()
    out = kernel(**inputs)
    print("kernel done in", time.time() - t0, "s", out.shape, out.dtype)
